# revision 5
# baseline (speedup 1.0000x reference)
"""GCN layer relu((A_hat @ x) @ W + b) on 8 TRN2 NeuronCores (Bass/Tile).

Self-contained: kernel(**inputs) takes FULL inputs, returns FULL output.

Strategy (dst-sharded SPMD, one program on 8 cores):
  - nodes split into 8 contiguous shards of 12500; per core, edges with dst
    in shard (plus self-loops) grouped by (window-group, src_chunk, window),
    a window being 128 consecutive dst nodes and a window-group being 8
    windows whose PSUM accumulators stay resident together.
  - each (chunk, window) region padded to 128-slot blocks using the max
    count across cores so one SPMD program fits all cores.
  - src features pulled by dma_gather (SWDGE indirect DMA, int16 indices
    per 25000-row chunk of x) as fp32, converted to fp16 by one batched
    ScalarE op per gather call.
  - per 128-slot block: one fused DVE tensor_scalar builds the norm-scaled
    one-hot [128, 128] in fp16 (pads have norm=0); TensorE accumulates
    psum[:, w] += G16.T @ S with a contiguous accumulation group per
    window (all chunks consecutive - fp16 matmul, 1 cycle/row).
  - per window: DVE copies psum -> SBUF fp16, TensorE computes
    psum2 = ones^T*b + aggT.T @ W16 (bias via K=1 matmul into PSUM),
    ScalarE applies ReLU, DMA writes the 128 output rows.
  - host concatenates the 8 shards. Host work is index/metadata prep only.
"""
import math

import numpy as np

import concourse.bacc as bacc
import concourse.mybir as mybir
import concourse.tile as tile
from concourse import bass_utils

P = 128
FEAT = 128
N_NODES = 100000
NCORES = 8
WINDOW = 128
CHUNK_ROWS = 25000
CALL_MAX_BLOCKS = 48
WGROUP = 8

NPC = N_NODES // NCORES                     # 12500 dst nodes per core
NW = math.ceil(NPC / WINDOW)                # 98 windows per core
DPAD = NW * WINDOW                          # 12544 padded dst rows per core
NCHUNK = math.ceil(N_NODES / CHUNK_ROWS)    # 4 src chunks
NWG = math.ceil(NW / WGROUP)                # 13 window-groups


def _host_prep(x, edge_index, edge_weight, W, b):
    n = N_NODES
    src = np.asarray(edge_index[0], dtype=np.int64)
    dst = np.asarray(edge_index[1], dtype=np.int64)
    ew = np.asarray(edge_weight, dtype=np.float32)
    loops = np.arange(n, dtype=np.int64)
    src = np.concatenate([src, loops])
    dst = np.concatenate([dst, loops])
    ew = np.concatenate([ew, np.ones(n, np.float32)])

    deg = np.bincount(dst, weights=ew.astype(np.float64), minlength=n)
    deg = deg.astype(np.float32)
    dinv = np.where(deg > 0, 1.0 / np.sqrt(deg), 0.0).astype(np.float32)
    norm = (dinv[src] * ew * dinv[dst]).astype(np.float32)

    core = dst // NPC
    dst_local = dst - core * NPC
    w_id = dst_local // WINDOW
    wg_id = w_id // WGROUP
    dst_in_w = (dst_local % WINDOW).astype(np.float32)
    c_id = src // CHUNK_ROWS
    idx_local = (src - c_id * CHUNK_ROWS).astype(np.int16)

    flat = (core * NCHUNK + c_id) * NW + w_id
    counts = np.bincount(flat, minlength=NCORES * NCHUNK * NW)
    counts = counts.reshape(NCORES, NCHUNK, NW)
    B = np.ceil(counts.max(axis=0) / P).astype(np.int64)  # [NCHUNK, NW]

    regions = []  # (c, w, blk0, nblk), block array ordered (wgroup, c, w)
    calls = []    # (c, blk0, nblk), each within one (wgroup, chunk)
    acc = 0
    for wg in range(NWG):
        ws = range(wg * WGROUP, min((wg + 1) * WGROUP, NW))
        for c in range(NCHUNK):
            span0 = acc
            for w in ws:
                regions.append((c, w, acc, int(B[c, w])))
                acc += int(B[c, w])
            k = span0
            while k < acc:
                nblk = min(CALL_MAX_BLOCKS, acc - k)
                calls.append((c, k, nblk))
                k += nblk
    nb_total = acc
    slots_total = nb_total * P

    wfirst, wlast = {}, {}
    for (c, w, blk0, nblk) in regions:
        if nblk == 0:
            continue
        if w not in wfirst:
            wfirst[w] = blk0
        wlast[w] = blk0 + nblk - 1
    assert len(wfirst) == NW

    meta = dict(regions=regions, calls=calls, nb_total=nb_total,
                slots_total=slots_total, wfirst=wfirst, wlast=wlast)

    order_all = np.lexsort((w_id, c_id, wg_id, core))
    core_sorted = core[order_all]
    core_starts = np.searchsorted(core_sorted, np.arange(NCORES + 1))
    x32 = np.ascontiguousarray(np.asarray(x, dtype=np.float32))
    W16 = np.ascontiguousarray(np.asarray(W, dtype=np.float16))
    b16 = np.asarray(b, dtype=np.float16).reshape(1, FEAT)
    ones16 = np.ones((1, P), dtype=np.float16)
    iota = np.tile(np.arange(WINDOW, dtype=np.float16)[None, :], (P, 1))

    in_maps = []
    for m in range(NCORES):
        sel = order_all[core_starts[m]:core_starts[m + 1]]
        midx, mdstw, mnorm = idx_local[sel], dst_in_w[sel], norm[sel]

        idx16 = np.zeros(slots_total, dtype=np.int16)
        dstloc = np.zeros(slots_total, dtype=np.float32)
        nrm = np.zeros(slots_total, dtype=np.float32)
        pos = 0
        for (c, w, blk0, nblk) in regions:
            cnt = int(counts[m, c, w])
            s0 = blk0 * P
            idx16[s0:s0 + cnt] = midx[pos:pos + cnt]
            dstloc[s0:s0 + cnt] = mdstw[pos:pos + cnt]
            nrm[s0:s0 + cnt] = mnorm[pos:pos + cnt]
            pos += cnt
        assert pos == len(sel)

        idx_tile = np.zeros((P, slots_total // 16), dtype=np.int16)
        for (c, blk0, nblk) in calls:
            s0, s1 = blk0 * P, (blk0 + nblk) * P
            seg = idx16[s0:s1].reshape(-1, 16).T
            idx_tile[:, s0 // 16:s1 // 16] = np.tile(seg, (8, 1))

        in_maps.append({
            "x": x32,
            "idx": idx_tile,
            "dstloc": dstloc.reshape(nb_total, P).T.copy(),
            "normt": nrm.reshape(nb_total, P).T.copy(),
            "iota": iota,
            "Wt": W16,
            "brow": b16,
            "ones1": ones16,
        })
    return meta, in_maps


def _build_kernel(meta):
    nb_total = meta["nb_total"]
    slots_total = meta["slots_total"]
    regions = meta["regions"]
    calls = meta["calls"]
    wfirst, wlast = meta["wfirst"], meta["wlast"]
    WD = WINDOW
    CAP = CALL_MAX_BLOCKS
    f16, f32 = mybir.dt.float16, mybir.dt.float32

    nc = bacc.Bacc("TRN2", target_bir_lowering=False, debug=False,
                   num_devices=NCORES, num_swdge_queues=4)
    x = nc.dram_tensor("x", [N_NODES, FEAT], f32, kind="ExternalInput")
    idx = nc.dram_tensor("idx", [P, slots_total // 16], mybir.dt.int16,
                         kind="ExternalInput")
    dstloc = nc.dram_tensor("dstloc", [P, nb_total], f32,
                            kind="ExternalInput")
    normt = nc.dram_tensor("normt", [P, nb_total], f32, kind="ExternalInput")
    iota = nc.dram_tensor("iota", [P, WD], f16, kind="ExternalInput")
    Wt = nc.dram_tensor("Wt", [FEAT, FEAT], f16, kind="ExternalInput")
    brow = nc.dram_tensor("brow", [1, FEAT], f16, kind="ExternalInput")
    ones1 = nc.dram_tensor("ones1", [1, P], f16, kind="ExternalInput")
    out = nc.dram_tensor("out", [DPAD, FEAT], f32, kind="ExternalOutput")

    call_of_block = {}
    for ci, (c, blk0, nblk) in enumerate(calls):
        for bb in range(blk0, blk0 + nblk):
            call_of_block[bb] = ci

    # regions grouped per window (c ascending) — matmuls are emitted
    # window-major so each PSUM slice's accumulation group is contiguous
    # (interleaved start/accumulate across slices loses partials on HW).
    regs_by_w = {}
    for (c, w, blk0, nblk) in regions:
        regs_by_w.setdefault(w, []).append((c, blk0, nblk))

    with tile.TileContext(nc) as tc:
        with (
            tc.tile_pool(name="const", bufs=1) as constp,
            tc.tile_pool(name="gbuf", bufs=2) as gbufp,
            tc.tile_pool(name="g16buf", bufs=6) as g16p,
            tc.tile_pool(name="sel", bufs=10) as selp,
            tc.tile_pool(name="aggw", bufs=6) as aggwp,
            tc.tile_pool(name="pswg", bufs=3, space="PSUM") as pswgp,
            tc.tile_pool(name="ps2", bufs=2, space="PSUM") as ps2p,
            tc.tile_pool(name="outst", bufs=4) as outp,
        ):
            idx_sb = constp.tile([P, slots_total // 16], mybir.dt.int16)
            dstloc_sb = constp.tile([P, nb_total], f32)
            normt_sb = constp.tile([P, nb_total], f32)
            iota_sb = constp.tile([P, WD], f16)
            W_sb = constp.tile([FEAT, FEAT], f16)
            b_sb = constp.tile([1, FEAT], f16)
            ones_sb = constp.tile([1, P], f16)

            nc.sync.dma_start(out=idx_sb[:], in_=idx[:])
            nc.sync.dma_start(out=dstloc_sb[:], in_=dstloc[:])
            nc.sync.dma_start(out=normt_sb[:], in_=normt[:])
            nc.sync.dma_start(out=iota_sb[:], in_=iota[:])
            nc.sync.dma_start(out=W_sb[:], in_=Wt[:])
            nc.sync.dma_start(out=b_sb[:], in_=brow[:])
            nc.sync.dma_start(out=ones_sb[:], in_=ones1[:])

            gtiles = {}
            issued = set()

            def gather_call(ci):
                c, blk0, nblk = calls[ci]
                g = gbufp.tile([P, CAP, FEAT], f32, tag="g")
                g16 = g16p.tile([P, CAP, FEAT], f16, tag="g16")
                nidx = nblk * P
                c0 = c * CHUNK_ROWS
                c1 = min(c0 + CHUNK_ROWS, N_NODES)
                nc.gpsimd.dma_gather(
                    g[:, :nblk, :], x[c0:c1, :],
                    idx_sb[:, blk0 * 8:(blk0 + nblk) * 8],
                    nidx, nidx, FEAT, single_packet=False,
                    queue_num=ci % 4,
                )
                nc.scalar.activation(
                    g16[:, :nblk, :], g[:, :nblk, :],
                    mybir.ActivationFunctionType.Copy,
                )
                gtiles[ci] = (g16, blk0)

            for wg in range(NWG):
                ws = list(range(wg * WGROUP, min((wg + 1) * WGROUP, NW)))
                ps = pswgp.tile([P, WGROUP, WD], f32, tag="ps")
                for w in ws:
                    wi = w - wg * WGROUP
                    for (c, blk0, nblk) in regs_by_w[w]:
                        if nblk == 0:
                            continue
                        for gb in range(blk0, blk0 + nblk):
                            ci = call_of_block[gb]
                            if ci not in issued:
                                gather_call(ci)
                                issued.add(ci)
                            g16, cblk0 = gtiles[ci]
                            col = gb - cblk0
                            st = selp.tile([P, WD], f16, tag="sel")
                            nc.vector.tensor_scalar(
                                out=st[:], in0=iota_sb[:],
                                scalar1=dstloc_sb[:, gb:gb + 1],
                                scalar2=normt_sb[:, gb:gb + 1],
                                op0=mybir.AluOpType.is_equal,
                                op1=mybir.AluOpType.mult,
                            )
                            nc.tensor.matmul(
                                out=ps[:, wi, :], lhsT=g16[:, col, :],
                                rhs=st[:],
                                start=(gb == wfirst[w]),
                                stop=(gb == wlast[w]),
                                skip_group_check=True,
                            )
                for w in ws:
                    wi = w - wg * WGROUP
                    aggw = aggwp.tile([P, WD], f16, tag="aggw")
                    nc.scalar.activation(
                        aggw[:], ps[:, wi, :],
                        mybir.ActivationFunctionType.Copy,
                    )
                    ps2 = ps2p.tile([WD, FEAT], f32, tag="ps2")
                    nc.tensor.matmul(
                        out=ps2[:], lhsT=ones_sb[:], rhs=b_sb[:],
                        start=True, stop=False, skip_group_check=True,
                    )
                    nc.tensor.matmul(
                        out=ps2[:], lhsT=aggw[:], rhs=W_sb[:],
                        start=False, stop=True, skip_group_check=True,
                    )
                    ot = outp.tile([WD, FEAT], f32, tag="ot")
                    nc.scalar.activation(
                        ot[:], ps2[:], mybir.ActivationFunctionType.Relu,
                    )
                    d0 = w * WD
                    nc.sync.dma_start(out=out[d0:d0 + WD, :], in_=ot[:])
    nc.compile()
    return nc


def kernel(x, edge_index, edge_weight, W, b):
    assert x.shape == (N_NODES, FEAT)
    meta, in_maps = _host_prep(x, edge_index, edge_weight, W, b)
    nc = _build_kernel(meta)
    res = bass_utils.run_bass_kernel_spmd(
        nc, in_maps, core_ids=list(range(NCORES)), trace=False)
    outs = [res.results[m]["out"][:NPC] for m in range(NCORES)]
    return np.ascontiguousarray(np.concatenate(outs, axis=0))
